# revision 5
# baseline (speedup 1.0000x reference)
"""Trainium2 Bass kernel for nn_Circuit_71330816852913.

Physics: B=512 independent optical-cavity mode vectors A(t) in C^64 obeying
    dA/dt = A @ G + i * nl^2 * |A|^2 (.) A,   G = T2^T + i*diag(omega)
integrated over t in [0,1], sampled at 200 evenly spaced points (h = 1/199).

Method (v2): KW=16 step windows. The backbone advances the state with ONE
Strang segment per window (linear flow exact via host-precomputed fp64 matrix
exponentials, applied as fp32 128x128 real-block matmuls; nonlinear flow is an
elementwise phase rotation approximated by cos x ~ 1 - x^2/2, sin x ~ x). The
15 intermediate outputs per window are INDEPENDENT one-shot Lie jumps
out_j = rot(phi_j) o (E(j h) @ Y) done in bf16: the matmuls run at 1 cyc/row
(4x over fp32) and the elementwise rotation runs as window-wide 960-column
"super ops" batched across all 15 branches, balanced across the ACT/DVE/Pool
engines. |V|^2 pair-sums use per-branch-scaled SUM2 matmuls on the otherwise
idle PE. The backbone phase is taken from branch j=8's phi (x2 scale), which
removes the backbone's own square/SUM2 round trip from the latency chain.
Outputs accumulate in a bf16 SBUF slab (halves DMA bytes) and the host widens
to fp32. Host-validated vs the fp64 adaptive reference: rel_l2 ~ 2.9e-3.

Sharding: pure data parallel over the batch dim, 64 rows per core on 8 cores.
On-chip layout per core: state stored transposed as [128 partitions, 64]
= [Re(A).T ; Im(A).T] (modes on partitions, batch on the free dim).
"""

import numpy as np

MODES = 64
INPUT_MODES = 48
LAMBD = 0.25
T_END = 1.0
EVAL_PTS = 200
N_CORES = 8
B_TOTAL = 512
BL = B_TOTAL // N_CORES  # 64 batch rows per core
NSTEP = EVAL_PTS - 1
H = T_END / NSTEP
KW = 16
NFULL = NSTEP // KW          # 12 full windows
KLAST = NSTEP - NFULL * KW   # 7-step final window
CHUNK = 40                   # slab flush granularity (steps)

# branch slice order within a full window: j=8 first (backbone phase source)
ORD = [8, 1, 2, 3, 4, 5, 6, 7, 9, 10, 11, 12, 13, 14, 15]

# bf16 weight stack: 0..14 -> E(j*h) for j=1..15 ; 15..29 -> (nl^2*j*h)*SUM2
NWBF = 30
# f32 weight stack: 0: E(8h), 1: SWAPS@E(8h), 2: E(3.5h), 3: SWAPS@E(3.5h),
# 4: (nl^2*7h)*SUM2
NWF = 5


def _build_G(omega, kappa, params):
    """G (complex128 [64,64]) such that the linear RHS is A @ G for row-batch A."""
    n = MODES
    k = n * (n - 1) // 2
    p = params.astype(np.float64)
    diag_p = p[: n - 1]
    re = p[n - 1 : n - 1 + k]
    im = p[n - 1 + k :]
    Hm = np.zeros((n, n), np.complex128)
    iu, ju = np.triu_indices(n, 1)
    Hm[iu, ju] = re + 1j * im
    Hm = Hm + Hm.conj().T
    Hm = Hm + np.diag(np.concatenate([diag_p, [-diag_p.sum()]]))
    w, V = np.linalg.eigh(Hm)
    U = (V * np.exp(1j * w)[None, :]) @ V.conj().T
    I = np.eye(n)
    UtU = U.T @ U
    mix = UtU @ np.linalg.inv(I * (1.0 + LAMBD) - UtU)
    kap2 = (kappa.astype(np.float64).astype(np.complex128)) ** 2
    sk = np.sqrt(kap2)
    T2 = -(sk[:, None] * (0.5 * I + mix) * sk[None, :])
    return T2.T + 1j * np.diag(omega.astype(np.float64))


def _expm_series(X, terms=30):
    n = X.shape[0]
    E = np.eye(n, dtype=X.dtype)
    term = np.eye(n, dtype=X.dtype)
    for k in range(1, terms):
        term = term @ X / k
        E = E + term
    return E


_PROGRAM = None


def _get_program():
    global _PROGRAM
    if _PROGRAM is not None:
        return _PROGRAM

    import concourse.bacc as bacc
    import concourse.mybir as mybir
    import concourse.tile as tile
    from contextlib import ExitStack

    f32 = mybir.dt.float32
    bf16 = mybir.dt.bfloat16
    Alu = mybir.AluOpType
    Act = mybir.ActivationFunctionType

    nc = bacc.Bacc(
        "TRN2", target_bir_lowering=False, debug=False, num_devices=N_CORES
    )
    y0_d = nc.declare_dram_parameter("y0", [128, 128], f32, isOutput=False)
    wbf_d = nc.declare_dram_parameter("wbf", [NWBF, 128, 128], bf16, isOutput=False)
    wf_d = nc.declare_dram_parameter("wf", [NWF, 128, 128], f32, isOutput=False)
    out_d = nc.declare_dram_parameter("out", [128, EVAL_PTS * BL], bf16, isOutput=True)

    RSQRT2 = float(1.0 / np.sqrt(2.0))
    SQRT2 = float(np.sqrt(2.0))

    with ExitStack() as ctx:
        tc = ctx.enter_context(tile.TileContext(nc))
        const = ctx.enter_context(tc.tile_pool(name="const", bufs=1))
        statep = ctx.enter_context(tc.tile_pool(name="statep", bufs=2))
        work = ctx.enter_context(tc.tile_pool(name="work", bufs=2))
        slabp = ctx.enter_context(tc.tile_pool(name="slab", bufs=1))
        vwp = ctx.enter_context(tc.tile_pool(name="vwp", bufs=1, space="PSUM"))
        phip = ctx.enter_context(tc.tile_pool(name="phip", bufs=1, space="PSUM"))
        pbp = ctx.enter_context(tc.tile_pool(name="pbp", bufs=2, space="PSUM"))

        wsbf = const.tile([128, NWBF * 128], bf16, tag="wsbf")
        for i in range(NWBF):
            nc.sync.dma_start(wsbf[:, i * 128 : (i + 1) * 128], wbf_d[i])
        wsf = const.tile([128, NWF * 128], f32, tag="wsf")
        for i in range(NWF):
            nc.sync.dma_start(wsf[:, i * 128 : (i + 1) * 128], wf_d[i])

        slab = slabp.tile([128, EVAL_PTS * BL], bf16, tag="slab")
        state0 = statep.tile([128, 128], f32, tag="state", name="state0")
        nc.sync.dma_start(state0[:], y0_d[:])

        def WE(j):  # bf16 E(j*h)
            return wsbf[:, (j - 1) * 128 : j * 128]

        def WS2(j):  # bf16 scaled SUM2 for branch j
            return wsbf[:, (15 + j - 1) * 128 : (15 + j) * 128]

        def WF(i):  # f32 slice
            return wsf[:, i * 128 : (i + 1) * 128]

        # pre-observe every weight-slice DMA on the PE
        scratch = pbp.tile([1, 1], f32, tag="pb", name="scratch")
        for i in range(NWBF):
            nc.tensor.matmul(
                scratch[:],
                wsbf[0:1, i * 128 : i * 128 + 1],
                wsbf[0:1, i * 128 : i * 128 + 1],
                start=True, stop=True,
            )
        for i in range(NWF):
            nc.tensor.matmul(
                scratch[:],
                wsf[0:1, i * 128 : i * 128 + 1],
                wsf[0:1, i * 128 : i * 128 + 1],
                start=True, stop=True,
            )

        # initial state mirror + slab[step 0]
        X0 = statep.tile([128, 128], bf16, tag="xmir", name="x0")
        nc.scalar.copy(X0[:], state0[:])
        nc.scalar.copy(slab[:, 0:BL], state0[:, 0:BL])

        next_chunk = [0]

        def flush_chunks(done_through):
            # steps 0..done_through are final in slab
            while (next_chunk[0] + 1) * CHUNK - 1 <= done_through:
                c = next_chunk[0]
                lo, hi = c * CHUNK * BL, (c + 1) * CHUNK * BL
                nc.sync.dma_start(out_d[:, lo:hi], slab[:, lo:hi])
                next_chunk[0] += 1

        Ystate, X = state0, X0
        uid = [0]
        t = 0
        while t < NSTEP:
            kk = min(KW, NSTEP - t)
            u = uid[0]
            uid[0] += 1
            full = kk == KW
            nb = kk - 1  # branch count
            order = ORD if full else list(range(1, kk))

            VW = vwp.tile([128, 15 * 128], f32, tag="vw", name=f"vw{u}")
            PHI = phip.tile([128, 960], f32, tag="phi", name=f"phi{u}")
            PB = pbp.tile([128, 128], f32, tag="pb", name=f"pb{u}")
            PN = pbp.tile([128, 128], f32, tag="pb", name=f"pn{u}")

            # --- PE: branch linear flows (bf16) + backbone fh1 (f32) ---
            nc.tensor.matmul(
                VW[:, 0:128], WE(order[0]), X[:], start=True, stop=True
            )
            nc.tensor.matmul(
                PB[:], WF(0) if full else WF(2), Ystate[:], start=True, stop=True
            )
            # S for slice 0 (ACT), then its SUM2 (feeds the backbone on full windows)
            Sb = work.tile([128, 960], bf16, tag="Sb", name=f"Sb{u}")
            nc.scalar.activation(Sb[:, 0:64], VW[:, 0:64], Act.Square, 0.0, 1.0)
            nc.tensor.matmul(
                PHI[:, 0:64], WS2(order[0]), Sb[:, 0:64], start=True, stop=True
            )

            # --- remaining branch matmuls first: keeps the PE queue dense ---
            for i in range(1, nb):
                nc.tensor.matmul(
                    VW[:, i * 128 : (i + 1) * 128],
                    WE(order[i]), X[:], start=True, stop=True,
                )
            # S super for slices 1..nb-1 (strided V-halves)
            if nb > 1:
                vw3 = VW[:, 128 : nb * 128].rearrange("p (j c) -> p j c", c=128)
                sb3 = Sb[:, 64 : nb * 64].rearrange("p (j c) -> p j c", c=64)
                nc.scalar.activation(sb3, vw3[:, :, 0:64], Act.Square, 0.0, 1.0)
            for i in range(1, nb):
                nc.tensor.matmul(
                    PHI[:, i * 64 : (i + 1) * 64],
                    WS2(order[i]),
                    Sb[:, i * 64 : (i + 1) * 64],
                    start=True, stop=True,
                )

            # phi copies to SBUF (HW allows only one PSUM operand per op)
            n64 = nb * 64
            phis = work.tile([128, 960], bf16, tag="phis", name=f"phis{u}")
            nc.vector.tensor_copy(phis[:, 0:n64], PHI[:, 0:n64])
            phb = work.tile([128, 64], f32, tag="phb", name=f"phb{u}")

            # backbone nonlinear rotation (quad cos, sin~x), f32
            qb = work.tile([128, 64], f32, tag="qb", name=f"qb{u}")
            t2b = work.tile([128, 64], f32, tag="t2b", name=f"t2b{u}")
            bbb = work.tile([128, 64], f32, tag="bbb", name=f"bbb{u}")
            nlob = work.tile([128, 64], f32, tag="nlob", name=f"nlob{u}")
            if full:
                # phase = 2 * phi_8 (bf16-derived, validated)
                nc.vector.tensor_copy(phb[:], PHI[:, 0:64])
                nc.scalar.activation(qb[:], PHI[:, 0:64], Act.Square, 0.0, SQRT2)
                nc.vector.scalar_tensor_tensor(
                    t2b[:], phb[:], 2.0, PB[:, 64:128], Alu.mult, Alu.mult
                )
            else:
                # f32 phase path for the odd-sized final window
                sbb = work.tile([128, 64], f32, tag="sbb", name=f"sbb{u}")
                nc.scalar.activation(sbb[:], PB[:, 0:64], Act.Square, 0.0, 1.0)
                nc.tensor.matmul(
                    PHI[:, 896:960], WF(4), sbb[:], start=True, stop=True
                )
                nc.vector.tensor_copy(phb[:], PHI[:, 896:960])
                nc.scalar.activation(
                    qb[:], PHI[:, 896:960], Act.Square, 0.0, RSQRT2
                )
                nc.vector.tensor_tensor(
                    t2b[:], phb[:], PB[:, 64:128], Alu.mult
                )
            nc.vector.scalar_tensor_tensor(
                bbb[:], qb[:], 1.0, PB[:, 0:64], Alu.subtract, Alu.mult
            )
            nc.gpsimd.tensor_tensor(nlob[:], t2b[:], bbb[:], Alu.subtract)

            # backbone second half-step: rebuild [Y|iY] pair (or just Y at the end)
            nc.tensor.matmul(
                PN[:, 0:64], WF(0) if full else WF(2), nlob[:], start=True, stop=True
            )
            if full:
                nc.tensor.matmul(
                    PN[:, 64:128], WF(1), nlob[:], start=True, stop=True
                )
                Ynew = statep.tile([128, 128], f32, tag="state", name=f"st{u}")
                nc.scalar.copy(Ynew[:], PN[:])
                Xnew = statep.tile([128, 128], bf16, tag="xmir", name=f"x{u}")
                nc.scalar.copy(Xnew[:], PN[:])
            nc.scalar.copy(
                slab[:, (t + kk) * BL : (t + kk + 1) * BL], PN[:, 0:64]
            )

            # --- branch rotation supers ---
            qt = work.tile([128, 960], bf16, tag="qt", name=f"qt{u}")
            nc.scalar.activation(
                qt[:, 0:n64], PHI[:, 0:n64], Act.Square, 0.0, RSQRT2
            )
            vwa = VW[:, 0 : nb * 128].rearrange("p (j c) -> p j c", c=128)
            phi3 = phis[:, 0:n64].rearrange("p (j c) -> p j c", c=64)
            t2s = work.tile([128, 960], bf16, tag="t2s", name=f"t2s{u}")
            t2s3 = t2s[:, 0:n64].rearrange("p (j c) -> p j c", c=64)
            nc.vector.tensor_tensor(t2s3, phi3, vwa[:, :, 64:128], Alu.mult)
            bbs = work.tile([128, 960], bf16, tag="bbs", name=f"bbs{u}")
            bbs3 = bbs[:, 0:n64].rearrange("p (j c) -> p j c", c=64)
            qt3 = qt[:, 0:n64].rearrange("p (j c) -> p j c", c=64)
            nc.vector.scalar_tensor_tensor(
                bbs3, qt3, 1.0, vwa[:, :, 0:64], Alu.subtract, Alu.mult
            )
            # out = t2 - bb -> slab (bf16, 2x DVE)
            if full:
                nc.gpsimd.tensor_tensor(
                    slab[:, (t + 8) * BL : (t + 9) * BL],
                    t2s[:, 0:64], bbs[:, 0:64], Alu.subtract,
                )
                nc.gpsimd.tensor_tensor(
                    slab[:, (t + 1) * BL : (t + 8) * BL],
                    t2s[:, 64:512], bbs[:, 64:512], Alu.subtract,
                )
                nc.gpsimd.tensor_tensor(
                    slab[:, (t + 9) * BL : (t + 16) * BL],
                    t2s[:, 512:960], bbs[:, 512:960], Alu.subtract,
                )
            else:
                nc.gpsimd.tensor_tensor(
                    slab[:, (t + 1) * BL : (t + kk) * BL],
                    t2s[:, 0:n64], bbs[:, 0:n64], Alu.subtract,
                )

            if full:
                Ystate, X = Ynew, Xnew
            t += kk
            flush_chunks(t)

    nc.finalize()
    _PROGRAM = nc
    return nc


def kernel(A0_real, A0_imag, omega, kappa, nonlinearity, params):
    from concourse.bass_utils import run_bass_kernel_spmd
    import ml_dtypes

    bf = ml_dtypes.bfloat16

    A0_real = np.asarray(A0_real, np.float32)
    A0_imag = np.asarray(A0_imag, np.float32)
    omega = np.asarray(omega, np.float32)
    kappa = np.asarray(kappa, np.float32)
    nonlinearity = np.asarray(nonlinearity, np.float32)
    params = np.asarray(params, np.float32)

    G = _build_G(omega, kappa, params)
    GT = G.T
    I64 = np.eye(64)
    Z64 = np.zeros((64, 64))
    SWAPS = np.block([[Z64, -I64], [I64, Z64]])
    SUM2 = np.block([[I64, I64], [I64, I64]])

    def real_block(C):
        return np.block([[C.real, -C.imag], [C.imag, C.real]])

    def lhsT(M):
        return np.ascontiguousarray(M.T)

    nl = float(nonlinearity.reshape(-1)[0])
    nl2 = nl * nl

    wbf = np.zeros((NWBF, 128, 128), bf)
    for j in range(1, 16):
        E = real_block(_expm_series(j * H * GT))
        wbf[j - 1] = lhsT(E).astype(bf)
        wbf[15 + j - 1] = lhsT((nl2 * j * H) * SUM2).astype(bf)

    wf = np.zeros((NWF, 128, 128), np.float32)
    Eh16 = real_block(_expm_series(0.5 * KW * H * GT))
    Eh7 = real_block(_expm_series(0.5 * KLAST * H * GT))
    wf[0] = lhsT(Eh16).astype(np.float32)
    wf[1] = lhsT(SWAPS @ Eh16).astype(np.float32)
    wf[2] = lhsT(Eh7).astype(np.float32)
    wf[3] = lhsT(SWAPS @ Eh7).astype(np.float32)
    wf[4] = lhsT((nl2 * KLAST * H) * SUM2).astype(np.float32)

    Ar = np.concatenate(
        [A0_real, np.ones((B_TOTAL, MODES - INPUT_MODES), np.float32)], axis=1
    )
    Ai = np.concatenate(
        [A0_imag, np.zeros((B_TOTAL, MODES - INPUT_MODES), np.float32)], axis=1
    )
    Y0 = np.concatenate([Ar.T, Ai.T], axis=0).astype(np.float32)  # [128, 512]
    Y0sw = np.concatenate([-Y0[64:128], Y0[0:64]], axis=0).astype(np.float32)

    nc = _get_program()
    in_maps = []
    for c in range(N_CORES):
        in_maps.append(
            {
                "y0": np.ascontiguousarray(
                    np.concatenate(
                        [
                            Y0[:, c * BL : (c + 1) * BL],
                            Y0sw[:, c * BL : (c + 1) * BL],
                        ],
                        axis=1,
                    )
                ),
                "wbf": wbf,
                "wf": wf,
            }
        )
    res = run_bass_kernel_spmd(nc, in_maps, list(range(N_CORES)))

    parts = []
    for c in range(N_CORES):
        arr = np.asarray(res.results[c]["out"]).astype(np.float32)
        parts.append(arr.reshape(2, 64, EVAL_PTS, BL).transpose(2, 0, 3, 1))
    out = np.concatenate(parts, axis=2)  # [200, 2, 512, 64]
    return np.ascontiguousarray(out.astype(np.float32))
